# revision 20
# baseline (speedup 1.0000x reference)
"""Multi-head attention (B=4, N=2048, EMB=768, H=8, D=96) on 8 TRN2 NeuronCores.

Sharding: core c -> batch b = c//2, head group = 4 heads (c%2)*4 .. (c%2)*4+3.
Each core computes the qkv projection for its batch restricted to its heads,
full-sequence attention for those heads, and a partial output projection.
Host sums the two partials per batch and adds the effective bias.

All matmuls run in float32r (TF32-like, 1 cycle/row at free dim >= 256).
Softmax skips the per-row max-subtraction: a global constant SHIFT keeps exp
arguments below ~48 (raw scores reach ~88, right at fp32 exp overflow), and
softmax is invariant to a uniform shift.  The k-projection bias is dropped
entirely (it shifts each softmax row uniformly), and the v bias is folded
into the host-side output bias (softmax rows sum to one).  Row sums come
free from a constant 1/INV_SCALE column appended to v inside the attn@v
matmul, so the reciprocal of the sum is already scaled by INV_SCALE.

Scheduling: one in-order PE stream of 128 "slots" (16 attention windows of
(head, 512-query block) x 8 key-chunk pairs).  Each slot issues a scores
pair, the attn@v pair lagged by two slots (so the exp on the Scalar engine
is never on the PE critical path), and filler projection work (q/k/v/out
projections) drawn from a deadline-ordered queue that keeps the PE dense.
"""
import math
from contextlib import ExitStack

import ml_dtypes
import numpy as np

import concourse.bass as bass
import concourse.tile as tile
from concourse import bacc, mybir
from concourse.bass_utils import run_bass_kernel_spmd

F32 = mybir.dt.float32
F32R = mybir.dt.float32r
BF16 = mybir.dt.bfloat16
AF = mybir.ActivationFunctionType
ALU = mybir.AluOpType

B, N, EMB, H, D = 4, 2048, 768, 8, 96
HPC = 4                      # heads per core
NCORES = 8
INV_SCALE = 1.0 / math.sqrt(D)
SHIFT = 44.0                 # global exp-argument shift (see module docstring)
EC = EMB // 128              # 6 contraction chunks over emb
IB = N // 512                # 4 query blocks of 512
JC = N // 128                # 16 key chunks of 128
PACE = 1400.0                # target emitted-PE-ns per slot for filler pacing

_cache = {}


def _build(reps=1):
    nc = bacc.Bacc("TRN2", target_bir_lowering=False, debug=False,
                   num_devices=NCORES)
    # host-packed inputs: one DMA per tensor, e-chunks side by side on the
    # free dim so each lands as a single 128 x contiguous-row transfer
    xb = [nc.dram_tensor(f"x{b}", [128, EC * 512], F32R,
                         kind="ExternalInput").ap() for b in range(IB)]
    wk0 = nc.dram_tensor("wk0", [128, EC * D], F32R, kind="ExternalInput").ap()
    wq0 = nc.dram_tensor("wq0", [128, EC * D], F32R, kind="ExternalInput").ap()
    wqkr = nc.dram_tensor("wqkr", [128, EC * 6 * D], F32R,
                          kind="ExternalInput").ap()
    wv = nc.dram_tensor("wv", [128, EC * HPC * D], F32R,
                        kind="ExternalInput").ap()
    bqd = nc.dram_tensor("bq", [D, HPC], F32, kind="ExternalInput").ap()
    wp = nc.dram_tensor("wp", [128, 3 * EMB], F32R, kind="ExternalInput").ap()
    onesd = nc.dram_tensor("ones", [128, HPC], BF16, kind="ExternalInput").ap()
    y = nc.dram_tensor("y", [EMB, N], F32, kind="ExternalOutput").ap()

    with tile.TileContext(nc) as tc, ExitStack() as ctx:
        xp = ctx.enter_context(tc.tile_pool(name="xp", bufs=EC * IB))
        wqp = ctx.enter_context(tc.tile_pool(name="wqp", bufs=1))
        wvp = ctx.enter_context(tc.tile_pool(name="wvp", bufs=1))
        wpp = ctx.enter_context(tc.tile_pool(name="wpp", bufs=1))
        qkp = ctx.enter_context(tc.tile_pool(name="qkp", bufs=4))
        vp = ctx.enter_context(tc.tile_pool(name="vp", bufs=JC))
        ep = ctx.enter_context(tc.tile_pool(name="ep", bufs=6))
        yhp = ctx.enter_context(tc.tile_pool(name="yhp", bufs=HPC))
        ysp = ctx.enter_context(tc.tile_pool(name="ysp", bufs=3))
        rp = ctx.enter_context(tc.tile_pool(name="rp", bufs=2))
        sp = ctx.enter_context(tc.tile_pool(name="sp", bufs=1))
        pp = ctx.enter_context(tc.tile_pool(name="pp", bufs=3, space="PSUM"))
        acc = ctx.enter_context(tc.tile_pool(name="acc", bufs=2, space="PSUM"))

        def body():
            # ---- DMA loads, priority-ordered ----------------------------
            # sync: smalls, then x blocks 0..2.  scalar: wqk (before its exp
            # stream starts).  gpsimd: wv, x block 3, wp.  vector: none (it
            # must run q-bias/k-copy compute immediately).
            shiftb = sp.tile([128, 1], F32, tag="shiftb")
            nc.vector.memset(shiftb[:], -SHIFT)

            # sync: x blocks 0,1.  scalar: head-0 q/k weights, smalls, rest
            # of wqk (ahead of its exp stream).  gpsimd: wv, x2, x3, wp.
            wk0t = wqp.tile([128, EC * D], F32R, tag="wk0")
            nc.scalar.dma_start(out=wk0t[:], in_=wk0[:])
            wq0t = wqp.tile([128, EC * D], F32R, tag="wq0")
            nc.scalar.dma_start(out=wq0t[:], in_=wq0[:])

            # x tiles stay per-(e, block) so the e-accumulations pipeline
            # with the DMA stream instead of waiting on one big semaphore.
            # Startup-critical bytes balanced across the three DMA queues:
            # sync x0+x1, scalar wk0/wq0+x2, gpsimd wv+x3.
            xt = [[None] * IB for _ in range(EC)]
            wvat = wvp.tile([128, EC * HPC * D], F32R, tag="wv")
            nc.gpsimd.dma_start(out=wvat[:], in_=wv[:])
            for b, eng in ((0, nc.sync), (2, nc.scalar), (1, nc.sync),
                           (3, nc.gpsimd)):
                for e in range(EC):
                    t = xp.tile([128, 512], F32R, tag="x", name=f"x{e}b{b}")
                    eng.dma_start(out=t[:],
                                  in_=xb[b][:, 512 * e:512 * (e + 1)])
                    xt[e][b] = t
            bqt = sp.tile([D, HPC], F32, tag="bq")
            nc.scalar.dma_start(out=bqt[:], in_=bqd[:])
            ones4 = sp.tile([128, HPC, 1], BF16, tag="ones4")
            nc.scalar.dma_start(out=ones4[:], in_=onesd[:])
            wqkrt = wqp.tile([128, EC * 6 * D], F32R, tag="wqkr")
            nc.scalar.dma_start(out=wqkrt[:], in_=wqkr[:])
            wpat = wpp.tile([128, 3 * EMB], F32R, tag="wp")
            nc.gpsimd.dma_start(out=wpat[:], in_=wp[:])
            wvt = [wvat[:, HPC * D * e:HPC * D * (e + 1)] for e in range(EC)]
            wpd = [wpat[:, EMB * dc:EMB * (dc + 1)] for dc in range(3)]

            def wq_ap(h, e):
                if h == 0:
                    return wq0t[:, D * e:D * (e + 1)]
                return wqkrt[:, 6 * D * e + D * (h - 1):6 * D * e + D * h]

            def wk_ap(h, e):
                if h == 0:
                    return wk0t[:, D * e:D * (e + 1)]
                return wqkrt[:, 6 * D * e + 3 * D + D * (h - 1):
                             6 * D * e + 3 * D + D * h]

            # ---- work units --------------------------------------------
            kt = [None] * HPC
            qt = [None] * HPC
            vt = [None] * JC
            yhq = [yhp.tile([128, N], F32R, tag="yh", name=f"yhq{i}")
                   for i in range(3)]
            # head h of the packed [384, N] yh lives at rows 96h..96h+95,
            # split across the three 128-partition tiles
            # PSUM reads at a nonzero partition offset must stay inside a
            # 32-partition group, so offset reads are split into 32-chunks
            YH_SPLITS = {0: ((0, 96, 0, 0),),
                         1: ((0, 32, 0, 96), (32, 64, 1, 0), (64, 96, 1, 32)),
                         2: ((0, 32, 1, 64), (32, 64, 1, 96), (64, 96, 2, 0)),
                         3: ((0, 32, 2, 32), (32, 64, 2, 64), (64, 96, 2, 96))}

            def k_unit(h, b):
                if kt[h] is None:
                    kt[h] = qkp.tile([D, N], F32R, tag="qk", name=f"kt{h}")
                pq = pp.tile([128, 1024], F32, tag="ps")
                for e in range(EC):
                    nc.tensor.matmul(
                        out=pq[:D, 0:512],
                        lhsT=wk_ap(h, e),
                        rhs=xt[e][b], start=(e == 0), stop=(e == EC - 1))
                nc.vector.tensor_copy(
                    out=kt[h][:, 512 * b:512 * (b + 1)], in_=pq[:D, 0:512])

            def q_unit(h, b):
                if qt[h] is None:
                    qt[h] = qkp.tile([D, N], F32R, tag="qk", name=f"qt{h}")
                pq = pp.tile([128, 1024], F32, tag="ps")
                for e in range(EC):
                    nc.tensor.matmul(
                        out=pq[:D, 0:512],
                        lhsT=wq_ap(h, e),
                        rhs=xt[e][b], start=(e == 0), stop=(e == EC - 1))
                nc.vector.tensor_scalar(
                    out=qt[h][:, 512 * b:512 * (b + 1)], in0=pq[:D, 0:512],
                    scalar1=bqt[:, h:h + 1], scalar2=None, op0=ALU.add)

            def v_unit(j):
                b, c = j // 4, j % 4
                pv = pp.tile([128, 1024], F32, tag="ps")
                for e in range(EC):
                    nc.tensor.matmul(
                        out=pv[:, 0:HPC * D],
                        lhsT=xt[e][b][:, 128 * c:128 * (c + 1)],
                        rhs=wvt[e], start=(e == 0), stop=(e == EC - 1))
                t = vp.tile([128, HPC, D + 1], BF16, tag="v")
                nc.vector.tensor_copy(
                    out=t[:, :, 0:D],
                    in_=pv[:, 0:HPC * D].rearrange("p (h d) -> p h d", h=HPC))
                nc.vector.tensor_copy(out=t[:, :, D:D + 1], in_=ones4[:])
                vt[j] = t

            def proj_partial(w, o):
                py = pp.tile([128, 1024], F32, tag="ps", name=f"py{w}_{o}")
                for dc in range(2):
                    nc.tensor.matmul(
                        out=py[:, 0:512],
                        lhsT=wpd[dc][:, 128 * o:128 * (o + 1)],
                        rhs=yhq[dc][:, 512 * w:512 * (w + 1)],
                        start=(dc == 0), stop=False)
                return py

            def proj_final(w, o, py):
                nc.tensor.matmul(
                    out=py[:, 0:512],
                    lhsT=wpd[2][:, 128 * o:128 * (o + 1)],
                    rhs=yhq[2][:, 512 * w:512 * (w + 1)],
                    start=False, stop=True)
                ys = ysp.tile([128, 512], F32, tag="ys")
                nc.vector.tensor_copy(out=ys[:], in_=py[:, 0:512])
                nc.sync.dma_start(
                    out=y[128 * o:128 * (o + 1), 512 * w:512 * (w + 1)],
                    in_=ys[:])

            def proj_unit(w, o):
                proj_final(w, o, proj_partial(w, o))

            # ---- filler queue: (unit, deadline_slot, earliest_slot, ns) --
            KQ, VU, PJ = 1280.0, 960.0, 1280.0
            fillers = []
            for j in range(2, JC):
                fillers.append((lambda j=j: v_unit(j), j // 2 + 1, 0, VU))
            for b in range(1, IB):
                fillers.append((lambda b=b: k_unit(0, b), 2 * b - 1, 0, KQ))
            for h in range(1, HPC):
                for b in range(IB):
                    fillers.append((lambda h=h, b=b: k_unit(h, b),
                                    32 * h + max(2 * b - 1, 0), 0, KQ))
            for h in range(HPC):
                for b in range(1 if h == 0 else 0, IB):
                    fillers.append((lambda h=h, b=b: q_unit(h, b),
                                    32 * h + 8 * b - 1, 0, KQ))
            for w in range(3):
                for o in range(6):
                    fillers.append((lambda w=w, o=o: proj_unit(w, o), 10 ** 9,
                                    8 * (12 + w) + 11, 660.0))
            fillers.sort(key=lambda u: u[1])

            emitted = [0.0]

            def pop_filler(slot, force_deadline):
                for idx, (fn, dl, es, ns) in enumerate(fillers):
                    if force_deadline and dl > slot:
                        return False
                    if es <= slot:
                        fillers.pop(idx)
                        fn()
                        emitted[0] += ns
                        return True
                    if force_deadline:
                        return False
                return False

            # ---- prefix -------------------------------------------------
            k_unit(0, 0)
            q_unit(0, 0)
            v_unit(0)
            v_unit(1)
            emitted[0] += 2 * KQ + 2 * VU

            # ---- main slot loop ----------------------------------------
            pav_fifo = []

            for h in range(HPC):
                for i4 in range(IB):
                    pav = acc.tile([D + 1, 512], F32, tag="pav")

                    def post(pav=pav, h=h, i4=i4):
                        # reciprocal_approx_fast needs an SBUF input (its
                        # bit-level seed breaks on the PSUM read path)
                        sums = rp.tile([1, 512], F32, tag="sums")
                        nc.vector.tensor_copy(out=sums[:], in_=pav[D:D + 1, :])
                        rec = rp.tile([1, 512], F32, tag="rec")
                        nc.vector.reciprocal_approx_fast(
                            out=rec[:], in_=sums[:])
                        recs = rp.tile([D, 512], F32, tag="recs")
                        nc.gpsimd.partition_broadcast(
                            recs[:], rec[0:1, :], channels=D)
                        for d0, d1, ti, r0 in YH_SPLITS[h]:
                            nc.vector.tensor_tensor(
                                out=yhq[ti][r0:r0 + d1 - d0,
                                            512 * i4:512 * (i4 + 1)],
                                in0=pav[d0:d1, :], in1=recs[d0:d1, :],
                                op=ALU.mult)

                    for j2 in range(JC // 2):
                        slot = 32 * h + 8 * i4 + j2
                        while pop_filler(slot, True):
                            pass
                        ps = pp.tile([128, 1024], F32, tag="ps")
                        for s in range(2):
                            j = 2 * j2 + s
                            nc.tensor.matmul(
                                out=ps[:, 512 * s:512 * (s + 1)],
                                lhsT=kt[h][:, 128 * j:128 * (j + 1)],
                                rhs=qt[h][:, 512 * i4:512 * (i4 + 1)],
                                start=True, stop=True)
                        et = ep.tile([128, 1024], BF16, tag="e")
                        nc.scalar.activation(out=et[:], in_=ps[:],
                                             func=AF.Exp, bias=shiftb[:])
                        emitted[0] += 430.0

                        def pav_pair(pav=pav, h=h, j2=j2, et=et):
                            for s in range(2):
                                j = 2 * j2 + s
                                nc.tensor.matmul(
                                    out=pav[:],
                                    lhsT=vt[j][:, h, :],
                                    rhs=et[:, 512 * s:512 * (s + 1)],
                                    start=(j == 0), stop=(j == JC - 1))
                            emitted[0] += 430.0

                        pav_fifo.append(pav_pair)
                        if j2 == JC // 2 - 1:
                            pav_fifo.append(post)
                        while len(pav_fifo) > 2:
                            pav_fifo.pop(0)()
                        while (emitted[0] < (slot + 1) * PACE
                               and pop_filler(slot, False)):
                            pass

            # ---- drain --------------------------------------------------
            # remaining pav pairs + post(3,3), then the last query block's
            # projection with the head-3 matmuls deferred so the PE keeps
            # streaming while post(3,3)'s DVE/gpsimd chain completes
            for fn in pav_fifo:
                fn()
            while pop_filler(10 ** 9, False):
                pass
            pyt = {o: proj_partial(3, o) for o in range(3)}
            for o in range(3, 6):
                proj_final(3, o - 3, pyt[o - 3])
                pyt[o] = proj_partial(3, o)
            for o in range(3, 6):
                proj_final(3, o, pyt[o])

        for _rep in range(reps):
            body()

    nc.compile()
    return nc


def _pack_e(a):
    """[EMB, cols] -> [128, EC*cols]: e-chunks side by side on the free dim."""
    cols = a.shape[1]
    return np.ascontiguousarray(
        a.reshape(EC, 128, cols).transpose(1, 0, 2).reshape(128, EC * cols),
        dtype=np.float32)


def _prep_in_maps(x, w_qkv, b_qkv, w_proj):
    wq = np.ascontiguousarray(w_qkv.reshape(EMB, H, D, 3))
    bq = np.ascontiguousarray(b_qkv.reshape(H, D, 3))
    in_maps = []
    for c in range(NCORES):
        b = c // 2
        h0 = (c % 2) * HPC
        hs = slice(h0, h0 + HPC)
        xTb = np.ascontiguousarray(x[b].T)
        bqc = np.stack([bq[h0 + h, :, 0] for h in range(HPC)], axis=1)
        wqkr = np.concatenate(
            [wq[:, h0 + 1:h0 + HPC, :, 0].reshape(EMB, 3 * D),
             wq[:, h0 + 1:h0 + HPC, :, 1].reshape(EMB, 3 * D)], axis=1)
        wpc = np.ascontiguousarray(
            w_proj.reshape(H, D, EMB)[hs].reshape(HPC * D, EMB))
        m = {
            "wk0": _pack_e(wq[:, h0, :, 1]),
            "wq0": _pack_e(wq[:, h0, :, 0]),
            "wqkr": _pack_e(wqkr),
            "wv": _pack_e(wq[:, hs, :, 2].reshape(EMB, HPC * D)),
            "bq": np.ascontiguousarray(bqc, dtype=np.float32),
            "wp": np.ascontiguousarray(
                (INV_SCALE * wpc).reshape(3, 128, EMB).transpose(1, 0, 2)
                .reshape(128, 3 * EMB), dtype=np.float32),
            "ones": np.ones((128, HPC), dtype=ml_dtypes.bfloat16),
        }
        for bb in range(IB):
            m[f"x{bb}"] = _pack_e(xTb[:, 512 * bb:512 * (bb + 1)])
        in_maps.append(m)
    return in_maps


def _run(x, w_qkv, b_qkv, w_proj, b_proj, trace=False):
    if "nc" not in _cache:
        _cache["nc"] = _build()
    x = np.asarray(x, dtype=np.float32)
    w_qkv = np.asarray(w_qkv, dtype=np.float32)
    b_qkv = np.asarray(b_qkv, dtype=np.float32)
    w_proj = np.asarray(w_proj, dtype=np.float32)
    b_proj = np.asarray(b_proj, dtype=np.float32)
    in_maps = _prep_in_maps(x, w_qkv, b_qkv, w_proj)
    res = run_bass_kernel_spmd(_cache["nc"], in_maps, list(range(NCORES)),
                               trace=trace)
    # v-bias contribution folds into a constant output row (softmax rows
    # sum to one): b_eff = b_proj + inv_scale * (b_v @ w_proj)
    bv_flat = b_qkv.reshape(H, D, 3)[:, :, 2].reshape(EMB)
    b_eff = b_proj + INV_SCALE * (bv_flat @ w_proj)
    out = np.empty((B, N, EMB), dtype=np.float32)
    for b in range(B):
        out[b] = (res.results[2 * b]["y"]
                  + res.results[2 * b + 1]["y"]).T + b_eff
    return out, res


def kernel(x, w_qkv, b_qkv, w_proj, b_proj):
    out, _ = _run(x, w_qkv, b_qkv, w_proj, b_proj, trace=False)
    return out


# revision 21
# speedup vs baseline: 1.0308x; 1.0308x over previous
"""Multi-head attention (B=4, N=2048, EMB=768, H=8, D=96) on 8 TRN2 NeuronCores.

Sharding: core c -> batch b = c//2, head group = 4 heads (c%2)*4 .. (c%2)*4+3.
Each core computes the qkv projection for its batch restricted to its heads,
full-sequence attention for those heads, and a partial output projection.
Host sums the two partials per batch and adds the effective bias.

All matmuls run in float32r (TF32-like, 1 cycle/row at free dim >= 256).
Softmax skips the per-row max-subtraction: a global constant SHIFT keeps exp
arguments below ~48 (raw scores reach ~88, right at fp32 exp overflow), and
softmax is invariant to a uniform shift.  The k-projection bias is dropped
entirely (it shifts each softmax row uniformly), and the v bias is folded
into the host-side output bias (softmax rows sum to one).  Row sums come
free from a constant 1/INV_SCALE column appended to v inside the attn@v
matmul, so the reciprocal of the sum is already scaled by INV_SCALE.

Scheduling: one in-order PE stream of 128 "slots" (16 attention windows of
(head, 512-query block) x 8 key-chunk pairs).  Each slot issues a scores
pair, the attn@v pair lagged by two slots (so the exp on the Scalar engine
is never on the PE critical path), and filler projection work (q/k/v/out
projections) drawn from a deadline-ordered queue that keeps the PE dense.
"""
import math
from contextlib import ExitStack

import ml_dtypes
import numpy as np

import concourse.bass as bass
import concourse.tile as tile
from concourse import bacc, mybir
from concourse.bass_utils import run_bass_kernel_spmd

F32 = mybir.dt.float32
F32R = mybir.dt.float32r
BF16 = mybir.dt.bfloat16
AF = mybir.ActivationFunctionType
ALU = mybir.AluOpType

B, N, EMB, H, D = 4, 2048, 768, 8, 96
HPC = 4                      # heads per core
NCORES = 8
INV_SCALE = 1.0 / math.sqrt(D)
SHIFT = 44.0                 # global exp-argument shift (see module docstring)
EC = EMB // 128              # 6 contraction chunks over emb
IB = N // 512                # 4 query blocks of 512
JC = N // 128                # 16 key chunks of 128
PACE = 1400.0                # target emitted-PE-ns per slot for filler pacing

_cache = {}


def _build(reps=1):
    nc = bacc.Bacc("TRN2", target_bir_lowering=False, debug=False,
                   num_devices=NCORES)
    # host-packed inputs: one DMA per tensor, e-chunks side by side on the
    # free dim so each lands as a single 128 x contiguous-row transfer
    xb = [nc.dram_tensor(f"x{b}", [128, EC * 512], F32R,
                         kind="ExternalInput").ap() for b in range(IB)]
    wk0 = nc.dram_tensor("wk0", [128, EC * D], F32R, kind="ExternalInput").ap()
    wq0 = nc.dram_tensor("wq0", [128, EC * D], F32R, kind="ExternalInput").ap()
    wqkr = nc.dram_tensor("wqkr", [128, EC * 6 * D], F32R,
                          kind="ExternalInput").ap()
    wv = nc.dram_tensor("wv", [128, EC * HPC * D], F32R,
                        kind="ExternalInput").ap()
    bqd = nc.dram_tensor("bq", [D, HPC], F32, kind="ExternalInput").ap()
    wp = nc.dram_tensor("wp", [D, HPC * EMB], F32R, kind="ExternalInput").ap()
    onesd = nc.dram_tensor("ones", [128, HPC], BF16, kind="ExternalInput").ap()
    y = nc.dram_tensor("y", [N, EMB], F32, kind="ExternalOutput").ap()

    with tile.TileContext(nc) as tc, ExitStack() as ctx:
        xp = ctx.enter_context(tc.tile_pool(name="xp", bufs=EC * IB))
        wqp = ctx.enter_context(tc.tile_pool(name="wqp", bufs=1))
        wvp = ctx.enter_context(tc.tile_pool(name="wvp", bufs=1))
        wpp = ctx.enter_context(tc.tile_pool(name="wpp", bufs=1))
        qkp = ctx.enter_context(tc.tile_pool(name="qkp", bufs=4))
        vp = ctx.enter_context(tc.tile_pool(name="vp", bufs=JC))
        ep = ctx.enter_context(tc.tile_pool(name="ep", bufs=6))
        yhp = ctx.enter_context(tc.tile_pool(name="yhp", bufs=HPC))
        ysp = ctx.enter_context(tc.tile_pool(name="ysp", bufs=3))
        rp = ctx.enter_context(tc.tile_pool(name="rp", bufs=2))
        sp = ctx.enter_context(tc.tile_pool(name="sp", bufs=1))
        pp = ctx.enter_context(tc.tile_pool(name="pp", bufs=3, space="PSUM"))
        acc = ctx.enter_context(tc.tile_pool(name="acc", bufs=2, space="PSUM"))

        def body():
            # ---- DMA loads, priority-ordered ----------------------------
            # sync: smalls, then x blocks 0..2.  scalar: wqk (before its exp
            # stream starts).  gpsimd: wv, x block 3, wp.  vector: none (it
            # must run q-bias/k-copy compute immediately).
            shiftb = sp.tile([128, 1], F32, tag="shiftb")
            nc.vector.memset(shiftb[:], -SHIFT)

            # sync: x blocks 0,1.  scalar: head-0 q/k weights, smalls, rest
            # of wqk (ahead of its exp stream).  gpsimd: wv, x2, x3, wp.
            wk0t = wqp.tile([128, EC * D], F32R, tag="wk0")
            nc.scalar.dma_start(out=wk0t[:], in_=wk0[:])
            wq0t = wqp.tile([128, EC * D], F32R, tag="wq0")
            nc.scalar.dma_start(out=wq0t[:], in_=wq0[:])

            # x tiles stay per-(e, block) so the e-accumulations pipeline
            # with the DMA stream instead of waiting on one big semaphore.
            # Startup-critical bytes balanced across the three DMA queues:
            # sync x0+x1, scalar wk0/wq0+x2, gpsimd wv+x3.
            xt = [[None] * IB for _ in range(EC)]
            wvat = wvp.tile([128, EC * HPC * D], F32R, tag="wv")
            nc.gpsimd.dma_start(out=wvat[:], in_=wv[:])
            for b, eng in ((0, nc.sync), (2, nc.scalar), (1, nc.sync),
                           (3, nc.gpsimd)):
                for e in range(EC):
                    t = xp.tile([128, 512], F32R, tag="x", name=f"x{e}b{b}")
                    eng.dma_start(out=t[:],
                                  in_=xb[b][:, 512 * e:512 * (e + 1)])
                    xt[e][b] = t
            bqt = sp.tile([D, HPC], F32, tag="bq")
            nc.scalar.dma_start(out=bqt[:], in_=bqd[:])
            ones4 = sp.tile([128, HPC, 1], BF16, tag="ones4")
            nc.scalar.dma_start(out=ones4[:], in_=onesd[:])
            wqkrt = wqp.tile([128, EC * 6 * D], F32R, tag="wqkr")
            nc.scalar.dma_start(out=wqkrt[:], in_=wqkr[:])
            wpat = wpp.tile([D, HPC * EMB], F32R, tag="wp")
            nc.gpsimd.dma_start(out=wpat[:], in_=wp[:])
            wvt = [wvat[:, HPC * D * e:HPC * D * (e + 1)] for e in range(EC)]
            wpt = [wpat[:, EMB * h:EMB * (h + 1)] for h in range(HPC)]

            def wq_ap(h, e):
                if h == 0:
                    return wq0t[:, D * e:D * (e + 1)]
                return wqkrt[:, 6 * D * e + D * (h - 1):6 * D * e + D * h]

            def wk_ap(h, e):
                if h == 0:
                    return wk0t[:, D * e:D * (e + 1)]
                return wqkrt[:, 6 * D * e + 3 * D + D * (h - 1):
                             6 * D * e + 3 * D + D * h]

            # ---- work units --------------------------------------------
            kt = [None] * HPC
            qt = [None] * HPC
            yht = [None] * HPC
            vt = [None] * JC

            def k_unit(h, b):
                if kt[h] is None:
                    kt[h] = qkp.tile([D, N], F32R, tag="qk", name=f"kt{h}")
                pq = pp.tile([128, 1024], F32, tag="ps")
                for e in range(EC):
                    nc.tensor.matmul(
                        out=pq[:D, 0:512],
                        lhsT=wk_ap(h, e),
                        rhs=xt[e][b], start=(e == 0), stop=(e == EC - 1))
                nc.vector.tensor_copy(
                    out=kt[h][:, 512 * b:512 * (b + 1)], in_=pq[:D, 0:512])

            def q_unit(h, b):
                if qt[h] is None:
                    qt[h] = qkp.tile([D, N], F32R, tag="qk", name=f"qt{h}")
                pq = pp.tile([128, 1024], F32, tag="ps")
                for e in range(EC):
                    nc.tensor.matmul(
                        out=pq[:D, 0:512],
                        lhsT=wq_ap(h, e),
                        rhs=xt[e][b], start=(e == 0), stop=(e == EC - 1))
                nc.vector.tensor_scalar(
                    out=qt[h][:, 512 * b:512 * (b + 1)], in0=pq[:D, 0:512],
                    scalar1=bqt[:, h:h + 1], scalar2=None, op0=ALU.add)

            def v_unit(j):
                b, c = j // 4, j % 4
                pv = pp.tile([128, 1024], F32, tag="ps")
                for e in range(EC):
                    nc.tensor.matmul(
                        out=pv[:, 0:HPC * D],
                        lhsT=xt[e][b][:, 128 * c:128 * (c + 1)],
                        rhs=wvt[e], start=(e == 0), stop=(e == EC - 1))
                t = vp.tile([128, HPC, D + 1], BF16, tag="v")
                nc.vector.tensor_copy(
                    out=t[:, :, 0:D],
                    in_=pv[:, 0:HPC * D].rearrange("p (h d) -> p h d", h=HPC))
                nc.vector.tensor_copy(out=t[:, :, D:D + 1], in_=ones4[:])
                vt[j] = t

            def proj_partial(i):
                py = pp.tile([128, 1024], F32, tag="ps", name=f"py{i}")
                for o0, ow in ((0, 512), (512, 256)):
                    for hh in range(HPC - 1):
                        nc.tensor.matmul(
                            out=py[:, o0:o0 + ow],
                            lhsT=yht[hh][:, 128 * i:128 * (i + 1)],
                            rhs=wpt[hh][:, o0:o0 + ow],
                            start=(hh == 0), stop=False)
                return py

            def proj_final(i, py):
                for o0, ow in ((0, 512), (512, 256)):
                    nc.tensor.matmul(
                        out=py[:, o0:o0 + ow],
                        lhsT=yht[HPC - 1][:, 128 * i:128 * (i + 1)],
                        rhs=wpt[HPC - 1][:, o0:o0 + ow],
                        start=False, stop=True)
                ys = ysp.tile([128, EMB], F32, tag="ys")
                nc.vector.tensor_copy(out=ys[:], in_=py[:, 0:EMB])
                nc.sync.dma_start(out=y[128 * i:128 * (i + 1), :], in_=ys[:])

            def proj_unit(i):
                proj_final(i, proj_partial(i))

            # ---- filler queue: (unit, deadline_slot, earliest_slot, ns) --
            KQ, VU, PJ = 1280.0, 960.0, 1280.0
            fillers = []
            for j in range(2, JC):
                fillers.append((lambda j=j: v_unit(j), j // 2 + 1, 0, VU))
            for b in range(1, IB):
                fillers.append((lambda b=b: k_unit(0, b), 2 * b - 1, 0, KQ))
            for h in range(1, HPC):
                for b in range(IB):
                    fillers.append((lambda h=h, b=b: k_unit(h, b),
                                    32 * h + max(2 * b - 1, 0), 0, KQ))
            for h in range(HPC):
                for b in range(1 if h == 0 else 0, IB):
                    fillers.append((lambda h=h, b=b: q_unit(h, b),
                                    32 * h + 8 * b - 1, 0, KQ))
            for i in range(12):
                fillers.append((lambda i=i: proj_unit(i), 10 ** 9,
                                8 * (12 + i // 4) + 11, PJ))
            fillers.sort(key=lambda u: u[1])

            emitted = [0.0]

            def pop_filler(slot, force_deadline):
                for idx, (fn, dl, es, ns) in enumerate(fillers):
                    if force_deadline and dl > slot:
                        return False
                    if es <= slot:
                        fillers.pop(idx)
                        fn()
                        emitted[0] += ns
                        return True
                    if force_deadline:
                        return False
                return False

            # ---- prefix -------------------------------------------------
            k_unit(0, 0)
            q_unit(0, 0)
            v_unit(0)
            v_unit(1)
            emitted[0] += 2 * KQ + 2 * VU

            # ---- main slot loop ----------------------------------------
            pav_fifo = []

            for h in range(HPC):
                for i4 in range(IB):
                    pav = acc.tile([D + 1, 512], F32, tag="pav")

                    def post(pav=pav, h=h, i4=i4):
                        # reciprocal_approx_fast needs an SBUF input (its
                        # bit-level seed breaks on the PSUM read path)
                        sums = rp.tile([1, 512], F32, tag="sums")
                        nc.vector.tensor_copy(out=sums[:], in_=pav[D:D + 1, :])
                        rec = rp.tile([1, 512], F32, tag="rec")
                        nc.vector.reciprocal_approx_fast(
                            out=rec[:], in_=sums[:])
                        recs = rp.tile([D, 512], F32, tag="recs")
                        nc.gpsimd.partition_broadcast(
                            recs[:], rec[0:1, :], channels=D)
                        if yht[h] is None:
                            yht[h] = yhp.tile([D, N], F32R, tag="yh",
                                              name=f"yh{h}")
                        nc.vector.tensor_tensor(
                            out=yht[h][:, 512 * i4:512 * (i4 + 1)],
                            in0=pav[0:D, :], in1=recs[:], op=ALU.mult)

                    for j2 in range(JC // 2):
                        slot = 32 * h + 8 * i4 + j2
                        while pop_filler(slot, True):
                            pass
                        ps = pp.tile([128, 1024], F32, tag="ps")
                        for s in range(2):
                            j = 2 * j2 + s
                            nc.tensor.matmul(
                                out=ps[:, 512 * s:512 * (s + 1)],
                                lhsT=kt[h][:, 128 * j:128 * (j + 1)],
                                rhs=qt[h][:, 512 * i4:512 * (i4 + 1)],
                                start=True, stop=True)
                        et = ep.tile([128, 1024], BF16, tag="e")
                        nc.scalar.activation(out=et[:], in_=ps[:],
                                             func=AF.Exp, bias=shiftb[:])
                        emitted[0] += 430.0

                        def pav_pair(pav=pav, h=h, j2=j2, et=et):
                            for s in range(2):
                                j = 2 * j2 + s
                                nc.tensor.matmul(
                                    out=pav[:],
                                    lhsT=vt[j][:, h, :],
                                    rhs=et[:, 512 * s:512 * (s + 1)],
                                    start=(j == 0), stop=(j == JC - 1))
                            emitted[0] += 430.0

                        pav_fifo.append(pav_pair)
                        if j2 == JC // 2 - 1:
                            pav_fifo.append(post)
                        while len(pav_fifo) > 2:
                            pav_fifo.pop(0)()
                        while (emitted[0] < (slot + 1) * PACE
                               and pop_filler(slot, False)):
                            pass

            # ---- drain --------------------------------------------------
            # remaining pav pairs + post(3,3), then the last query block's
            # projection with the head-3 matmuls deferred so the PE keeps
            # streaming while post(3,3)'s DVE/gpsimd chain completes
            for fn in pav_fifo:
                fn()
            while pop_filler(10 ** 9, False):
                pass
            pyt = {i: proj_partial(i) for i in range(12, 15)}
            proj_final(12, pyt[12])
            pyt[15] = proj_partial(15)
            for i in range(13, 16):
                proj_final(i, pyt[i])

        for _rep in range(reps):
            body()

    nc.compile()
    return nc


def _pack_e(a):
    """[EMB, cols] -> [128, EC*cols]: e-chunks side by side on the free dim."""
    cols = a.shape[1]
    return np.ascontiguousarray(
        a.reshape(EC, 128, cols).transpose(1, 0, 2).reshape(128, EC * cols),
        dtype=np.float32)


def _prep_in_maps(x, w_qkv, b_qkv, w_proj):
    wq = np.ascontiguousarray(w_qkv.reshape(EMB, H, D, 3))
    bq = np.ascontiguousarray(b_qkv.reshape(H, D, 3))
    in_maps = []
    for c in range(NCORES):
        b = c // 2
        h0 = (c % 2) * HPC
        hs = slice(h0, h0 + HPC)
        xTb = np.ascontiguousarray(x[b].T)
        bqc = np.stack([bq[h0 + h, :, 0] for h in range(HPC)], axis=1)
        wqkr = np.concatenate(
            [wq[:, h0 + 1:h0 + HPC, :, 0].reshape(EMB, 3 * D),
             wq[:, h0 + 1:h0 + HPC, :, 1].reshape(EMB, 3 * D)], axis=1)
        wpc = np.ascontiguousarray(
            w_proj.reshape(H, D, EMB)[hs].reshape(HPC * D, EMB))
        m = {
            "wk0": _pack_e(wq[:, h0, :, 1]),
            "wq0": _pack_e(wq[:, h0, :, 0]),
            "wqkr": _pack_e(wqkr),
            "wv": _pack_e(wq[:, hs, :, 2].reshape(EMB, HPC * D)),
            "bq": np.ascontiguousarray(bqc, dtype=np.float32),
            "wp": np.ascontiguousarray(
                (INV_SCALE * wpc).reshape(HPC, D, EMB).transpose(1, 0, 2)
                .reshape(D, HPC * EMB), dtype=np.float32),
            "ones": np.ones((128, HPC), dtype=ml_dtypes.bfloat16),
        }
        for bb in range(IB):
            m[f"x{bb}"] = _pack_e(xTb[:, 512 * bb:512 * (bb + 1)])
        in_maps.append(m)
    return in_maps


def _run(x, w_qkv, b_qkv, w_proj, b_proj, trace=False):
    if "nc" not in _cache:
        _cache["nc"] = _build()
    x = np.asarray(x, dtype=np.float32)
    w_qkv = np.asarray(w_qkv, dtype=np.float32)
    b_qkv = np.asarray(b_qkv, dtype=np.float32)
    w_proj = np.asarray(w_proj, dtype=np.float32)
    b_proj = np.asarray(b_proj, dtype=np.float32)
    in_maps = _prep_in_maps(x, w_qkv, b_qkv, w_proj)
    res = run_bass_kernel_spmd(_cache["nc"], in_maps, list(range(NCORES)),
                               trace=trace)
    # v-bias contribution folds into a constant output row (softmax rows
    # sum to one): b_eff = b_proj + inv_scale * (b_v @ w_proj)
    bv_flat = b_qkv.reshape(H, D, 3)[:, :, 2].reshape(EMB)
    b_eff = b_proj + INV_SCALE * (bv_flat @ w_proj)
    out = np.empty((B, N, EMB), dtype=np.float32)
    for b in range(B):
        out[b] = res.results[2 * b]["y"] + res.results[2 * b + 1]["y"] + b_eff
    return out, res


def kernel(x, w_qkv, b_qkv, w_proj, b_proj):
    out, _ = _run(x, w_qkv, b_qkv, w_proj, b_proj, trace=False)
    return out
